# revision 5
# baseline (speedup 1.0000x reference)
"""Trainium2 Bass kernel for nn_AR_Transcriber: conv stack + 2-layer LSTM
AR decode, fp16 hi/lo pair arithmetic (fp32-grade), data-parallel over batch
across 8 NeuronCores."""
import numpy as np
import concourse.mybir as mybir
import concourse.bass as bass

import numpy as np
import concourse.mybir as mybir

F16 = mybir.dt.float16
F32 = mybir.dt.float32
AF = mybir.ActivationFunctionType
ALU = mybir.AluOpType
AX = mybir.AxisListType
LO = 4096.0


def split_pair(W):
    W = np.asarray(W, np.float32)
    W1 = W.astype(np.float16)
    W2 = ((W - W1.astype(np.float32)) * LO).astype(np.float16)
    return W1, W2


def pack_pair_img(W, n_k, n_m, m_width=128):
    """W [out,in] -> [128, n_m*n_k*2*m_width] fp16: block (m,k) holds
    [W1tile | W2tile] of W.T, m-major. Zero-padded."""
    out_dim, in_dim = W.shape
    W1, W2 = split_pair(W)
    img = np.zeros((128, n_m * n_k * 2 * m_width), np.float16)
    for m in range(n_m):
        for k in range(n_k):
            col = (m * n_k + k) * 2 * m_width
            r0, r1 = k * 128, min((k + 1) * 128, in_dim)
            c0, c1 = m * m_width, min((m + 1) * m_width, out_dim)
            img[0:r1 - r0, col:col + (c1 - c0)] = W1[c0:c1, r0:r1].T
            img[0:r1 - r0, col + m_width:col + m_width + (c1 - c0)] = W2[c0:c1, r0:r1].T
    return img


def col_major(v, n_m):
    return np.ascontiguousarray(np.asarray(v, np.float32).reshape(n_m, 128).T)


class HostPack2:
    def __init__(self, inputs):
        W_ih0 = np.asarray(inputs['w_ih0'], np.float32)
        self.Wa = np.ascontiguousarray(W_ih0[:, :768])
        Wp = W_ih0[:, 768:]
        Wp2 = np.zeros((3072, 256), np.float32)
        idx = np.arange(88)
        Wp2[:, idx] = Wp[:, idx * 2]            # e=0 -> k-tile 0, partitions 0..87
        Wp2[:, 128 + idx] = Wp[:, idx * 2 + 1]  # e=1 -> k-tile 1
        post = np.asarray(inputs['post_w'], np.float32).reshape(88, 5, 768)
        post_img = np.zeros((128, 5 * 6 * 2 * 88), np.float16)
        for c in range(5):
            pc = pack_pair_img(np.ascontiguousarray(post[:, c, :]), 6, 1, m_width=88)
            post_img[:, c * 1056:(c + 1) * 1056] = pc
        self.imgs = {
            'whh0': pack_pair_img(np.asarray(inputs['w_hh0'], np.float32), 6, 24),
            'wp': pack_pair_img(Wp2, 2, 24),
            'wih1': pack_pair_img(np.asarray(inputs['w_ih1'], np.float32), 6, 24),
            'whh1': pack_pair_img(np.asarray(inputs['w_hh1'], np.float32), 6, 24),
            'post': post_img,
        }
        self.b0 = (np.asarray(inputs['b_ih0'], np.float32)
                   + np.asarray(inputs['b_hh0'], np.float32)).astype(np.float32)
        self.b1_cm = col_major((np.asarray(inputs['b_ih1'], np.float32)
                                + np.asarray(inputs['b_hh1'], np.float32)), 24)
        self.postb = np.ascontiguousarray(
            np.asarray(inputs['post_b'], np.float32).reshape(88, 5))
        emb = np.asarray(inputs['emb'], np.float32)
        e1, e2 = split_pair(emb)
        self.embc = {}
        for e in range(2):
            self.embc[('hi', e)] = np.broadcast_to(
                e1[:, e].astype(np.float32), (88, 5)).copy()
            self.embc[('lo', e)] = np.broadcast_to(
                e2[:, e].astype(np.float32), (88, 5)).copy()
        self.revi = np.broadcast_to(
            np.array([4, 3, 2, 1, 0], np.float32), (88, 5)).copy()
        self.clsc = np.broadcast_to(
            np.array([0, 1, 2, 3, 4], np.float32), (88, 5)).copy()
        self.emb = emb


class StepTiles2:
    """Device tiles for the v2 AR loop (allocated once, bufs=1)."""
    def __init__(self, nc, pool, psp):
        t = lambda shape, dt, name: pool.tile(list(shape), dt, name=name)
        # resident pair weights
        self.whh0 = t((128, 36864), F16, "whh0")
        self.wp = t((128, 12288), F16, "wp")
        self.wih1 = t((128, 36864), F16, "wih1")
        self.post = t((128, 5280), F16, "post")
        # rhs tiles, col layout per k: [r2, r1, 0]
        # rz0: k=0..5 h1-pair, k=6..7 prev-pair; rz1: k=0..5 h2-pair
        self.rz0 = t((128, 24), F16, "rz0")
        self.rz1 = t((128, 18), F16, "rz1")
        # state
        self.c1 = t((128, 6), F32, "c1"); self.c2 = t((128, 6), F32, "c2")
        self.h1 = t((128, 6), F32, "h1"); self.h2 = t((128, 6), F32, "h2")
        # gate scratch
        self.za = t((128, 24), F32, "za"); self.zb = t((128, 24), F32, "zb")
        self.z0 = t((128, 24), F32, "z0"); self.z1 = t((128, 24), F32, "z1")
        self.t_if = t((128, 12), F32, "t_if"); self.sig_if = t((128, 12), F32, "sig_if")
        self.tg = t((128, 6), F32, "tg"); self.t_o = t((128, 6), F32, "t_o")
        self.sig_o = t((128, 6), F32, "sig_o")
        self.u = t((128, 6), F32, "u"); self.v = t((128, 6), F32, "v")
        self.tc_ = t((128, 6), F32, "tc_")
        # argmax scratch
        self.logits = t((88, 5), F32, "logits"); self.la = t((88, 5), F32, "la")
        self.mx = t((88, 1), F32, "mx"); self.eq = t((88, 5), F32, "eq")
        self.eq2 = t((88, 5), F32, "eq2")
        self.wrv = t((88, 5), F32, "wrv"); self.rr = t((88, 1), F32, "rr")
        self.idx = t((88, 1), F32, "idx"); self.ps = t((88, 1), F32, "ps")
        # consts
        self.b1c = t((128, 24), F32, "b1c")
        self.postbc = t((88, 5), F32, "postbc")
        self.revic = t((88, 5), F32, "revic")
        self.clscc = t((88, 5), F32, "clscc")
        self.embc = {(h, e): t((88, 5), F32, f"embc_{h}{e}")
                     for h in ('hi', 'lo') for e in range(2)}
        # psums: interleaved [lo|hi] per m; one accumulation group open per
        # bank at any time (per-m groups open/close contiguously per tile)
        self.z0psA = psp.tile([128, 48], F32, name="z0psA")   # whh0 part
        self.z0psB = psp.tile([128, 48], F32, name="z0psB")   # wp part
        self.z1psA = psp.tile([128, 48], F32, name="z1psA")   # whh1 part
        self.z1psB = psp.tile([128, 48], F32, name="z1psB")   # wih1 part
        self.lgps = psp.tile([88, 10], F32, name="lgps")


def _mm_pair(nc, ps_pair, img, blk_col, rz, k3, start, stop, m_width=128):
    """Two N=2 matmuls for one (m,k) pair-tile into ps_pair [P,2]=[lo,hi].
    W1 reads rz[k3:k3+2]=[r2,r1] -> [lo+=W1@r2, hi+=W1@r1];
    W2 reads rz[k3+1:k3+3]=[r1,0] -> [lo+=W2@r1, hi+=0]."""
    W1 = img[:, blk_col:blk_col + m_width]
    W2 = img[:, blk_col + m_width:blk_col + 2 * m_width]
    nc.tensor.matmul(ps_pair, W1, rz[:, k3:k3 + 2], start=start, stop=False)
    nc.tensor.matmul(ps_pair, W2, rz[:, k3 + 1:k3 + 3], start=False, stop=stop)


def emit_A(nc, st):
    """z0 whh0-part for the NEXT step -> z0psA (per-m group opens+closes)."""
    for m in range(24):
        pp = st.z0psA[:, 2 * m:2 * m + 2]
        for k in range(6):
            _mm_pair(nc, pp, st.whh0, (m * 6 + k) * 256, st.rz0, 3 * k,
                     start=(k == 0), stop=(k == 5))


def emit_A_wp(nc, st):
    """z0 wp-part for the NEXT step -> z0psB (per-m group opens+closes)."""
    for m in range(24):
        pp = st.z0psB[:, 2 * m:2 * m + 2]
        for k in range(2):
            _mm_pair(nc, pp, st.wp, (m * 2 + k) * 256, st.rz0, 3 * (6 + k),
                     start=(k == 0), stop=(k == 1))


def _emit_gate_tail(nc, st, z, c, h, rz):
    """Common gates: z [128,24] -> h, c updated; h pair into rz cols."""
    TT, TS = nc.vector.tensor_tensor, nc.vector.tensor_scalar
    nc.scalar.activation(st.t_if[:], z[:, 0:12], AF.Tanh, scale=0.5)
    TS(st.sig_if[:], st.t_if[:], 0.5, 0.5, ALU.mult, ALU.add)
    nc.scalar.activation(st.tg[:], z[:, 12:18], AF.Tanh)
    nc.scalar.activation(st.t_o[:], z[:, 18:24], AF.Tanh, scale=0.5)
    TS(st.sig_o[:], st.t_o[:], 0.5, 0.5, ALU.mult, ALU.add)
    TT(st.u[:], st.sig_if[:, 6:12], c[:], ALU.mult)
    TT(st.v[:], st.sig_if[:, 0:6], st.tg[:], ALU.mult)
    TT(c[:], st.u[:], st.v[:], ALU.add)
    nc.scalar.activation(st.tc_[:], c[:], AF.Tanh)
    TT(h[:], st.sig_o[:], st.tc_[:], ALU.mult)
    nc.vector.tensor_copy(rz[:, 1:18:3], h[:])
    TT(st.u[:], h[:], rz[:, 1:18:3], ALU.subtract)
    TS(rz[:, 0:18:3], st.u[:], LO, None, ALU.mult)


def emit_step2(nc, st, z0g_ap, out_col_ap, whh1_fn, last=False):
    """One software-pipelined AR iteration: gates/argmax of step i + z0 GEMVs
    for step i+1. whh1_fn(m) -> [128, 1536] fp16 tile ([W1|W2] x 6k)."""
    TT, TS = nc.vector.tensor_tensor, nc.vector.tensor_scalar

    # [DVE] gates0(i): combine z0psA/B pairs + z0a(+b0) -> h1(i) into rz0
    # (each op reads at most one PSUM input)
    TS(st.za[:], st.z0psA[:, 0:48:2], 1.0 / LO, None, ALU.mult)
    TT(st.za[:], st.za[:], st.z0psA[:, 1:48:2], ALU.add)
    TS(st.zb[:], st.z0psB[:, 0:48:2], 1.0 / LO, None, ALU.mult)
    TT(st.zb[:], st.zb[:], st.z0psB[:, 1:48:2], ALU.add)
    TT(st.za[:], st.za[:], z0g_ap, ALU.add)
    TT(st.z0[:], st.za[:], st.zb[:], ALU.add)
    _emit_gate_tail(nc, st, st.z0, st.c1, st.h1, st.rz0)

    # [PE] z1 = whh1 @ h2(i-1) + wih1 @ h1(i); one psum group per m.
    LEAD = 3
    def whh1_grp(m):
        g = whh1_fn(m)
        pp = st.z1psA[:, 2 * m:2 * m + 2]
        for k in range(6):
            _mm_pair(nc, pp, g, k * 256, st.rz1, 3 * k,
                     start=(k == 0), stop=(k == 5))

    def wih1_grp(m):
        pp = st.z1psB[:, 2 * m:2 * m + 2]
        for k in range(6):
            _mm_pair(nc, pp, st.wih1, (m * 6 + k) * 256, st.rz0, 3 * k,
                     start=(k == 0), stop=(k == 5))

    for m in range(LEAD):
        whh1_grp(m)
    for m in range(24):
        if m + LEAD < 24:
            whh1_grp(m + LEAD)
        wih1_grp(m)

    # [DVE] gates1(i): combine z1psA/B pairs + b1 -> h2(i) into rz1
    TS(st.za[:], st.z1psA[:, 0:48:2], 1.0 / LO, None, ALU.mult)
    TT(st.za[:], st.za[:], st.z1psA[:, 1:48:2], ALU.add)
    TS(st.zb[:], st.z1psB[:, 0:48:2], 1.0 / LO, None, ALU.mult)
    TT(st.zb[:], st.zb[:], st.z1psB[:, 1:48:2], ALU.add)
    TT(st.za[:], st.za[:], st.b1c[:], ALU.add)
    TT(st.z1[:], st.za[:], st.zb[:], ALU.add)
    _emit_gate_tail(nc, st, st.z1, st.c2, st.h2, st.rz1)

    # [PE] A-whh0 for step i+1 (overlaps gates1 completion)
    if not last:
        emit_A(nc, st)

    # [PE] logits: post @ h2-pair
    for c in range(5):
        pp = st.lgps[:, 2 * c:2 * c + 2]
        for k in range(6):
            _mm_pair(nc, pp, st.post, (c * 6 + k) * 176, st.rz1, 3 * k,
                     start=(k == 0), stop=(k == 5), m_width=88)

    # [DVE] argmax + prev-pair into rz0
    TS(st.la[:], st.lgps[:, 0:10:2], 1.0 / LO, None, ALU.mult)
    TT(st.la[:], st.la[:], st.lgps[:, 1:10:2], ALU.add)
    TT(st.logits[:], st.la[:], st.postbc[:], ALU.add)
    nc.vector.reduce_max(st.mx[:], st.logits[:], axis=AX.X)
    TS(st.eq[:], st.logits[:], st.mx[:, 0:1], None, ALU.is_equal)
    TT(st.wrv[:], st.eq[:], st.revic[:], ALU.mult)
    nc.vector.reduce_max(st.rr[:], st.wrv[:], axis=AX.X)
    TS(st.idx[:], st.rr[:], -1.0, 4.0, ALU.mult, ALU.add)
    nc.vector.tensor_copy(out_col_ap, st.idx[:])
    TS(st.eq2[:], st.clscc[:], st.idx[:, 0:1], None, ALU.is_equal)  # tie-proof
    for e in range(2):
        TT(st.wrv[:], st.eq2[:], st.embc[('hi', e)][:], ALU.mult)
        nc.vector.reduce_sum(st.ps[:], st.wrv[:], axis=AX.X)
        nc.vector.tensor_copy(st.rz0[0:88, 3 * (6 + e) + 1:3 * (6 + e) + 2], st.ps[:])
        TT(st.wrv[:], st.eq2[:], st.embc[('lo', e)][:], ALU.mult)
        nc.vector.reduce_sum(st.ps[:], st.wrv[:], axis=AX.X)
        nc.vector.tensor_copy(st.rz0[0:88, 3 * (6 + e):3 * (6 + e) + 1], st.ps[:])

    # [PE] A-wp for step i+1 (needs prev(i))
    if not last:
        emit_A_wp(nc, st)


def load_step_consts2(nc, st, dram):
    nc.sync.dma_start(out=st.whh0[:], in_=dram['whh0'][:])
    nc.sync.dma_start(out=st.wp[:], in_=dram['wp'][:])
    nc.sync.dma_start(out=st.wih1[:], in_=dram['wih1'][:])
    nc.sync.dma_start(out=st.post[:], in_=dram['post'][:])
    nc.sync.dma_start(out=st.b1c[:], in_=dram['b1c'][:])
    nc.sync.dma_start(out=st.postbc[:], in_=dram['postbc'][:])
    nc.sync.dma_start(out=st.revic[:], in_=dram['revic'][:])
    nc.sync.dma_start(out=st.clscc[:], in_=dram['clsc'][:])
    for h in ('hi', 'lo'):
        for e in range(2):
            nc.sync.dma_start(out=st.embc[(h, e)][:], in_=dram[f'embc_{h}{e}'][:])
    for tile in (st.c1, st.c2, st.rz0, st.rz1, st.h1, st.h2):
        nc.vector.memset(tile[:], 0.0)


def step_input_maps2(hp):
    m = {nm: hp.imgs[nm] for nm in ('whh0', 'wp', 'wih1', 'post', 'whh1')}
    m['b1c'] = hp.b1_cm
    m['postbc'] = hp.postb
    m['revic'] = hp.revi
    m['clsc'] = hp.clsc
    for h in ('hi', 'lo'):
        for e in range(2):
            m[f'embc_{h}{e}'] = hp.embc[(h, e)]
    return m


# ============== conv/fc/z0a ==============

BN_EPS = 1e-5
T = 512
TC = 32          # t-rows per chunk (fewer, bigger loop iterations)
NCH = T // TC    # 16 chunks


def fold_bn(cw, cb, g, b, m, v):
    scale = (np.asarray(g, np.float32) / np.sqrt(np.asarray(v, np.float32) + np.float32(BN_EPS))).astype(np.float32)
    w = (np.asarray(cw, np.float32) * scale[:, None, None, None]).astype(np.float32)
    bias = (np.asarray(cb, np.float32) * scale + np.asarray(b, np.float32)
            - np.asarray(m, np.float32) * scale).astype(np.float32)
    return w, bias


class ConvPack:
    def __init__(self, inputs):
        w1, b1 = fold_bn(inputs['conv1_w'], inputs['conv1_b'], inputs['bn1_g'],
                         inputs['bn1_b'], inputs['bn1_m'], inputs['bn1_v'])
        w2, b2 = fold_bn(inputs['conv2_w'], inputs['conv2_b'], inputs['bn2_g'],
                         inputs['bn2_b'], inputs['bn2_m'], inputs['bn2_v'])
        w3, b3 = fold_bn(inputs['conv3_w'], inputs['conv3_b'], inputs['bn3_g'],
                         inputs['bn3_b'], inputs['bn3_m'], inputs['bn3_v'])
        c1l = np.zeros((9, 48), np.float32)
        for dt in range(3):
            for df in range(3):
                c1l[3*dt+df] = w1[:, 0, dt, df]
        self.c1l_1, self.c1l_2 = split_pair(c1l)
        self.b1 = np.ascontiguousarray(b1.reshape(48, 1))
        c2a = np.zeros((48, 9 * 48), np.float16); c2b = np.zeros((48, 9 * 48), np.float16)
        c3a = np.zeros((48, 9 * 96), np.float16); c3b = np.zeros((48, 9 * 96), np.float16)
        for tap in range(9):
            dt, df = tap // 3, tap % 3
            a, bq = split_pair(np.ascontiguousarray(w2[:, :, dt, df].T))
            c2a[:, tap*48:(tap+1)*48] = a; c2b[:, tap*48:(tap+1)*48] = bq
            a, bq = split_pair(np.ascontiguousarray(w3[:, :, dt, df].T))
            c3a[:, tap*96:(tap+1)*96] = a; c3b[:, tap*96:(tap+1)*96] = bq
        self.c2l_1, self.c2l_2 = c2a, c2b
        self.c3l_1, self.c3l_2 = c3a, c3b
        self.b2 = np.ascontiguousarray(b2.reshape(48, 1))
        self.b3 = np.ascontiguousarray(b3.reshape(96, 1))
        fcw = np.asarray(inputs['fc_w'], np.float32)
        fcw2 = np.zeros((768, 57 * 128), np.float32)
        for f in range(57):
            fcw2[:, f * 128: f * 128 + 96] = fcw[:, np.arange(96) * 57 + f]
        img = np.zeros((128, 57 * 6 * 256), np.float16)
        W1, W2 = split_pair(fcw2)
        for f in range(57):
            for m in range(6):
                col = (f * 6 + m) * 256
                img[:, col:col+128] = W1[m*128:(m+1)*128, f*128:(f+1)*128].T
                img[:, col+128:col+256] = W2[m*128:(m+1)*128, f*128:(f+1)*128].T
        self.fcw_img = img
        self.fcb_pm = np.ascontiguousarray(np.asarray(inputs['fc_b'], np.float32).reshape(6, 128).T)

    @staticmethod
    def wa_stream_img(Wa):
        W1, W2 = split_pair(Wa)
        img = np.zeros((128, 24 * 6 * 256), np.float16)
        for m in range(24):
            for k in range(6):
                col = (m * 6 + k) * 256
                img[:, col:col+128] = W1[m*128:(m+1)*128, k*128:(k+1)*128].T
                img[:, col+128:col+256] = W2[m*128:(m+1)*128, k*128:(k+1)*128].T
        return img


def melpad_pair(mel_row):
    mp = np.zeros((514, 231), np.float32)
    mp[1:513, 1:230] = np.asarray(mel_row, np.float32)
    m1 = mp.astype(np.float16)
    m2 = ((mp - m1.astype(np.float32)) * LO).astype(np.float16)
    return m1, m2


def emit_zero_pads(nc, pool, scrs):
    """Zero pad borders of HBM scratches [(handle, C, H, W), ...]."""
    mx = max(max(h, w) for _, _, h, w in scrs)
    zt = pool.tile([128, mx], F16, name="zpad")
    nc.vector.memset(zt[:], 0.0)
    for scr, C, H, W in scrs:
        nc.sync.dma_start(out=scr[:, 0, :], in_=zt[:C, :W])
        nc.sync.dma_start(out=scr[:, H-1, :], in_=zt[:C, :W])
        nc.sync.dma_start(out=scr[:, :, 0], in_=zt[:C, :H])
        nc.sync.dma_start(out=scr[:, :, W-1], in_=zt[:C, :H])


def emit_conv1(nc, tc, pool, psp, dram):
    Fp = 231
    c1w1 = pool.tile([9, 48], F16, name="c1w1"); c1w2 = pool.tile([9, 48], F16, name="c1w2")
    b1t = pool.tile([48, 1], F32, name="b1t")
    nc.sync.dma_start(out=c1w1[:], in_=dram['c1l_1'][:])
    nc.sync.dma_start(out=c1w2[:], in_=dram['c1l_2'][:])
    nc.sync.dma_start(out=b1t[:], in_=dram['c1b'][:])
    NW = TC * Fp
    with tc.For_i(0, NCH, 1, name="c1loop") as ch:
        t0r = ch * TC
        P1 = pool.tile([9, NW], F16, name="P1", bufs=2)
        P2 = pool.tile([9, NW], F16, name="P2", bufs=2)
        for tap in range(9):
            dt, df = tap // 3, tap % 3
            w = Fp - df
            nc.sync.dma_start(
                out=P1[tap:tap+1, :].rearrange("a (i j) -> a i j", j=Fp)[:, :, 0:w],
                in_=dram['mel1'][bass.ds(t0r + dt, TC), df:Fp])
            nc.sync.dma_start(
                out=P2[tap:tap+1, :].rearrange("a (i j) -> a i j", j=Fp)[:, :, 0:w],
                in_=dram['mel2'][bass.ds(t0r + dt, TC), df:Fp])
        for wi in range(TC // 2):
            off = wi * 2 * Fp
            N = 2 * Fp
            ph = psp.tile([48, 462], F32, name="c1ph", bufs=2)
            pl = psp.tile([48, 462], F32, name="c1pl", bufs=2)
            nc.tensor.matmul(ph[:, :N], c1w1[:], P1[:, off:off+N], start=True, stop=True)
            nc.tensor.matmul(pl[:, :N], c1w1[:], P2[:, off:off+N], start=True, stop=False)
            nc.tensor.matmul(pl[:, :N], c1w2[:], P1[:, off:off+N], start=False, stop=True)
            mg = pool.tile([48, 462], F32, name="c1mg", bufs=2)
            nc.vector.tensor_scalar(mg[:, :N], pl[:, :N], 1.0 / LO, None, ALU.mult)
            nc.vector.tensor_tensor(mg[:, :N], mg[:, :N], ph[:, :N], ALU.add)
            rl = pool.tile([48, 462], F32, name="c1rl", bufs=2)
            nc.scalar.activation(rl[:, :N], mg[:, :N], AF.Relu, bias=b1t[:, 0:1])
            s1 = pool.tile([48, 462], F16, name="c1s1", bufs=2)
            s2 = pool.tile([48, 462], F16, name="c1s2", bufs=2)
            nc.vector.tensor_copy(s1[:, :N], rl[:, :N])
            nc.vector.tensor_tensor(mg[:, :N], rl[:, :N], s1[:, :N], ALU.subtract)
            nc.vector.tensor_scalar(s2[:, :N], mg[:, :N], LO, None, ALU.mult)
            for s, nm in ((s1, 'c1p1'), (s2, 'c1p2')):
                nc.sync.dma_start(
                    out=dram[nm][:, bass.ds(t0r + wi * 2 + 1, 2), 1:230],
                    in_=s.rearrange("c (i j) -> c i j", j=Fp)[:, 0:2, 0:229])


def emit_convN(nc, tc, pool, psp, dram, in_nm, wkey, Cin, Cout, Fin, sink):
    """conv2/3: input HBM pad-pair [Cin, 514, Fin+2]; 27 MMs per 2-row window;
    relu; pool w2; sink(nc, ch_reg, wi, po_view [Cout,2,Fo])."""
    Fp = Fin + 2
    N = 2 * Fp
    Fo = Fin // 2
    wt1 = pool.tile([48, 9 * Cout], F16, name=f"wt1{wkey}")
    wt2 = pool.tile([48, 9 * Cout], F16, name=f"wt2{wkey}")
    bt = pool.tile([Cout, 1], F32, name=f"bt{wkey}")
    nc.sync.dma_start(out=wt1[:Cin, :], in_=dram[wkey + '_1'][:])
    nc.sync.dma_start(out=wt2[:Cin, :], in_=dram[wkey + '_2'][:])
    nc.sync.dma_start(out=bt[:], in_=dram[wkey + 'b'][:])
    with tc.For_i(0, NCH, 1, name=f"loop{wkey}") as ch:
        t0r = ch * TC
        X1 = pool.tile([Cin, (TC + 2) * Fp + 2], F16, name="cnX1", bufs=2)
        X2 = pool.tile([Cin, (TC + 2) * Fp + 2], F16, name="cnX2", bufs=2)
        nc.sync.dma_start(out=X1[:, 0:(TC + 2) * Fp].rearrange("c (i j) -> c i j", j=Fp),
                          in_=dram[in_nm + '1'][:, bass.ds(t0r, TC + 2), :])
        nc.sync.dma_start(out=X2[:, 0:(TC + 2) * Fp].rearrange("c (i j) -> c i j", j=Fp),
                          in_=dram[in_nm + '2'][:, bass.ds(t0r, TC + 2), :])
        for wi in range(TC // 2):
            ph = psp.tile([Cout, 512], F32, name="cnph", bufs=2)
            pl = psp.tile([Cout, 512], F32, name="cnpl", bufs=2)
            for tap in range(9):
                dt, df = tap // 3, tap % 3
                off = (wi * 2 + dt) * Fp + df
                l1 = wt1[:Cin, tap*Cout:(tap+1)*Cout]
                l2 = wt2[:Cin, tap*Cout:(tap+1)*Cout]
                nc.tensor.matmul(ph[:, :N], l1, X1[:, off:off+N], start=(tap == 0), stop=(tap == 8))
                nc.tensor.matmul(pl[:, :N], l1, X2[:, off:off+N], start=(tap == 0), stop=False)
                nc.tensor.matmul(pl[:, :N], l2, X1[:, off:off+N], start=False, stop=(tap == 8))
            mg = pool.tile([Cout, 512], F32, name="cnmg", bufs=2)
            nc.vector.tensor_scalar(mg[:, :N], pl[:, :N], 1.0 / LO, None, ALU.mult)
            nc.vector.tensor_tensor(mg[:, :N], mg[:, :N], ph[:, :N], ALU.add)
            rl = pool.tile([Cout, 512], F32, name="cnrl", bufs=2)
            nc.scalar.activation(rl[:, :N], mg[:, :N], AF.Relu, bias=bt[:, 0:1])
            pv = rl[:, :N].rearrange("c (i j) -> c i j", j=Fp)
            po = pool.tile([Cout, 2 * Fo], F32, name="cnpo", bufs=2)
            pov = po.rearrange("c (i j) -> c i j", j=Fo)
            nc.vector.tensor_tensor(pov, pv[:, 0:2, 0:2*Fo:2], pv[:, 0:2, 1:1+2*Fo:2], ALU.max)
            sink(nc, t0r, wi, po, Fo)


def make_pad_sink(pool, dram, out_nm, Cout):
    def sink(nc, t0r, wi, po, Fo):
        N = 2 * Fo
        s1 = pool.tile([Cout, 256], F16, name="pds1", bufs=2)
        s2 = pool.tile([Cout, 256], F16, name="pds2", bufs=2)
        tmp = pool.tile([Cout, 256], F32, name="pdtmp", bufs=2)
        nc.vector.tensor_copy(s1[:, :N], po[:])
        nc.vector.tensor_tensor(tmp[:, :N], po[:], s1[:, :N], ALU.subtract)
        nc.vector.tensor_scalar(s2[:, :N], tmp[:, :N], LO, None, ALU.mult)
        nc.sync.dma_start(out=dram[out_nm + '1'][:, bass.ds(t0r + wi*2 + 1, 2), 1:1+Fo],
                          in_=s1[:, :N].rearrange("c (i j) -> c i j", j=Fo))
        nc.sync.dma_start(out=dram[out_nm + '2'][:, bass.ds(t0r + wi*2 + 1, 2), 1:1+Fo],
                          in_=s2[:, :N].rearrange("c (i j) -> c i j", j=Fo))
    return sink


def make_feat_sink(pool, feat1, feat2, Cout):
    def sink(nc, t0r, wi, po, Fo):
        N = 2 * Fo
        s1 = pool.tile([Cout, N], F16, name="fts1", bufs=2)
        s2 = pool.tile([Cout, N], F16, name="fts2", bufs=2)
        tmp = pool.tile([Cout, N], F32, name="fttmp", bufs=2)
        nc.vector.tensor_copy(s1[:], po[:, :N])
        nc.vector.tensor_tensor(tmp[:], po[:, :N], s1[:], ALU.subtract)
        nc.vector.tensor_scalar(s2[:], tmp[:], LO, None, ALU.mult)
        for s, ft in ((s1, feat1), (s2, feat2)):
            nc.vector.tensor_copy(
                ft.rearrange("c (f t) -> c f t", t=512)[0:Cout, :, bass.ds(t0r + wi*2, 2)],
                s.rearrange("c (i j) -> c j i", j=Fo))
    return sink


def emit_fc_z0a(nc, tc, pool, psp, spool, dram, feat1, feat2, b0c, ac1, ac2):
    fcbt = pool.tile([128, 6], F32, name="fcbt")
    nc.sync.dma_start(out=fcbt[:], in_=dram['fcb'][:])
    for half in range(2):
        for mi in range(3):
            m = half * 3 + mi
            ph = psp.tile([128, 512], F32, name=f"fch{mi}")
            pl = psp.tile([128, 512], F32, name=f"fcl{mi}")
            for f in range(57):
                g = spool.tile([128, 256], F16, name="fcg")
                nc.sync.dma_start(out=g[:], in_=dram['fcws'][:, (f*6+m)*256:(f*6+m+1)*256])
                nc.tensor.matmul(ph[:], g[:, 0:128], feat1[:, f*512:(f+1)*512],
                                 start=(f == 0), stop=(f == 56))
                nc.tensor.matmul(pl[:], g[:, 0:128], feat2[:, f*512:(f+1)*512],
                                 start=(f == 0), stop=False)
                nc.tensor.matmul(pl[:], g[:, 128:256], feat1[:, f*512:(f+1)*512],
                                 start=False, stop=(f == 56))
            mg = pool.tile([128, 512], F32, name="fcmg", bufs=2)
            nc.vector.tensor_scalar(mg[:], pl[:], 1.0 / LO, None, ALU.mult)
            nc.vector.tensor_tensor(mg[:], mg[:], ph[:], ALU.add)
            nc.vector.tensor_scalar(mg[:], mg[:], fcbt[:, m:m+1], None, ALU.add)
            nc.vector.tensor_copy(ac1[:, m*512:(m+1)*512], mg[:])
            nc.vector.tensor_tensor(mg[:], mg[:], ac1[:, m*512:(m+1)*512], ALU.subtract)
            nc.vector.tensor_scalar(ac2[:, m*512:(m+1)*512], mg[:], LO, None, ALU.mult)
    for m in range(24):
        ph = psp.tile([128, 512], F32, name="zah")
        pl = psp.tile([128, 512], F32, name="zal")
        for k in range(6):
            g = spool.tile([128, 256], F16, name="wag")
            nc.sync.dma_start(out=g[:], in_=dram['was'][:, (m*6+k)*256:(m*6+k+1)*256])
            nc.tensor.matmul(ph[:], g[:, 0:128], ac1[:, k*512:(k+1)*512],
                             start=(k == 0), stop=(k == 5))
            nc.tensor.matmul(pl[:], g[:, 0:128], ac2[:, k*512:(k+1)*512],
                             start=(k == 0), stop=False)
            nc.tensor.matmul(pl[:], g[:, 128:256], ac1[:, k*512:(k+1)*512],
                             start=False, stop=(k == 5))
        mg = pool.tile([128, 512], F32, name="zamg", bufs=2)
        nc.vector.tensor_scalar(mg[:], pl[:], 1.0 / LO, None, ALU.mult)
        nc.vector.tensor_tensor(mg[:], mg[:], ph[:], ALU.add)
        nc.vector.tensor_scalar(mg[:], mg[:], b0c[:, m:m+1], None, ALU.add)
        nc.sync.dma_start(
            out=dram['z0a'].rearrange("p (t q) -> p t q", q=24)[:, 0:512, m],
            in_=mg[:])


def conv_input_maps(cp, wa_img, mel_row, b0_cm):
    m1, m2 = melpad_pair(mel_row)
    return {
        'mel1': m1, 'mel2': m2, 'b0c': b0_cm,
        'c1l_1': cp.c1l_1, 'c1l_2': cp.c1l_2, 'c1b': cp.b1,
        'c2_1': cp.c2l_1, 'c2_2': cp.c2l_2, 'c2b': cp.b2,
        'c3_1': cp.c3l_1, 'c3_2': cp.c3l_2, 'c3b': cp.b3,
        'fcb': cp.fcb_pm,
    }


def declare_conv_drams(nc, scratch_kind="Internal"):
    """Input + scratch DRAM tensors for the conv phase. Returns dict."""
    d = {}
    ins = [('mel1', [514, 231], F16), ('mel2', [514, 231], F16),
           ('c1l_1', [9, 48], F16), ('c1l_2', [9, 48], F16), ('c1b', [48, 1], F32),
           ('c2_1', [48, 9*48], F16), ('c2_2', [48, 9*48], F16), ('c2b', [48, 1], F32),
           ('c3_1', [48, 9*96], F16), ('c3_2', [48, 9*96], F16), ('c3b', [96, 1], F32),
           ('fcb', [128, 6], F32), ('b0c', [128, 24], F32)]
    for nm, shp, dt in ins:
        d[nm] = nc.dram_tensor(nm, shp, dt, kind="ExternalInput")
    scr = [('c1p1', [48, 514, 231], F16), ('c1p2', [48, 514, 231], F16),
           ('c2p1', [48, 514, 116], F16), ('c2p2', [48, 514, 116], F16)]
    for nm, shp, dt in scr:
        d[nm] = nc.dram_tensor(nm, shp, dt, kind=scratch_kind)
    return d


def emit_conv_phase(nc, tc, dram, b0_cm):
    """Full conv+fc+z0a phase. b0_cm: [128, 24] host array for b0 (loaded here).
    Uses its own pools; returns nothing (z0a left in dram['z0a'])."""
    with tc.tile_pool(name="cvp", bufs=1) as pool, \
         tc.tile_pool(name="cvps", bufs=1, space="PSUM") as psp:
        emit_zero_pads(nc, pool, [(dram['c1p1'], 48, 514, 231), (dram['c1p2'], 48, 514, 231),
                                  (dram['c2p1'], 48, 514, 116), (dram['c2p2'], 48, 514, 116)])
        emit_conv1(nc, tc, pool, psp, dram)
        emit_convN(nc, tc, pool, psp, dram, 'c1p', 'c2', 48, 48, 229,
                   make_pad_sink(pool, dram, 'c2p', 48))
    with tc.tile_pool(name="cvp3", bufs=1) as pool:
        feat1 = pool.tile([128, 57*512], F16, name="feat1")
        feat2 = pool.tile([128, 57*512], F16, name="feat2")
        nc.vector.memset(feat1[:], 0.0)
        nc.vector.memset(feat2[:], 0.0)
        with tc.tile_pool(name="cvps3", bufs=1, space="PSUM") as psp3:
            emit_convN(nc, tc, pool, psp3, dram, 'c2p', 'c3', 48, 96, 114,
                       make_feat_sink(pool, feat1, feat2, 96))
        with tc.tile_pool(name="fcps", bufs=1, space="PSUM") as pspf, \
             tc.tile_pool(name="fcsp", bufs=4) as spool:
            b0ct = pool.tile([128, 24], F32, name="b0ct")
            nc.sync.dma_start(out=b0ct[:], in_=dram['b0c'][:])
            ac1 = pool.tile([128, 6*512], F16, name="ac1")
            ac2 = pool.tile([128, 6*512], F16, name="ac2")
            emit_fc_z0a(nc, tc, pool, pspf, spool, dram, feat1, feat2, b0ct, ac1, ac2)


# ====================== kernel assembly ======================
import time as _time
import concourse.bacc as _bacc
from concourse.tile import TileContext as _TileContext
from concourse import bass_utils as _bass_utils

_T = 512
NRES = 2

BLOB_SPEC = [('whh0', 36864), ('wp', 12288), ('wih1', 36864), ('whh1', 36864),
             ('post', 5280), ('fcws', 57 * 6 * 256), ('was', 24 * 6 * 256)]
BLOB_COLS = sum(n for _, n in BLOB_SPEC)


def build_nc(conv_loops=1, ar_loops=1):
    nc = _bacc.Bacc(num_devices=8)
    dram = declare_conv_drams(nc)
    dram['z0a'] = nc.dram_tensor('z0a', [128, 513 * 24], F32)
    sim_shapes = {
        'b1c': [128, 24], 'postbc': [88, 5], 'revic': [88, 5], 'clsc': [88, 5],
        'embc_hi0': [88, 5], 'embc_hi1': [88, 5],
        'embc_lo0': [88, 5], 'embc_lo1': [88, 5],
    }
    for k, shp in sim_shapes.items():
        dram[k] = nc.dram_tensor(k, shp, F32, kind="ExternalInput")
    wslice = nc.dram_tensor('wslice', [16, BLOB_COLS], F16, kind="ExternalInput")
    wbounce = nc.dram_tensor('wbounce', [16, BLOB_COLS], F16)
    wblob = nc.dram_tensor('wblob', [128, BLOB_COLS], F16, addr_space="Shared")
    off = 0
    for nm, ncols in BLOB_SPEC:
        dram[nm] = wblob[:, off:off + ncols]
        off += ncols
    out_d = nc.dram_tensor('out', [88, _T], F32, kind="ExternalOutput")

    with _TileContext(nc) as tc:
        nc.sync.dma_start(out=wbounce[:], in_=wslice[:])
        nc.gpsimd.collective_compute(
            "AllGather", mybir.AluOpType.bypass,
            ins=[wbounce[:]], outs=[wblob[:]],
            replica_groups=[list(range(8))])
        for _ in range(conv_loops):
            emit_conv_phase(nc, tc, dram, None)
        with tc.tile_pool(name="arp", bufs=1) as pool, \
             tc.tile_pool(name="arsp", bufs=4) as spool, \
             tc.tile_pool(name="arzp", bufs=2) as zpool, \
             tc.tile_pool(name="arps", bufs=1, space="PSUM") as psp:
            st = StepTiles2(nc, pool, psp)
            out_s = pool.tile([88, _T], F32, name="out_s")
            whh1r = pool.tile([128, NRES * 1536], F16, name="whh1r")
            load_step_consts2(nc, st, dram)
            nc.sync.dma_start(out=whh1r[:], in_=dram['whh1'][:, 0:NRES * 1536])

            def whh1_fn(m):
                if m < NRES:
                    return whh1r[:, m * 1536:(m + 1) * 1536]
                g = spool.tile([128, 1536], F16, name="whh1g")
                nc.sync.dma_start(out=g[:], in_=dram['whh1'][:, m * 1536:(m + 1) * 1536])
                return g[:]

            for rep in range(ar_loops):
                emit_A(nc, st)
                emit_A_wp(nc, st)
                with tc.For_i(0, _T, 1, hint_engines=(mybir.EngineType.PE,)) as i:
                    z0g = zpool.tile([128, 24], F32, name="z0g")
                    nc.sync.dma_start(out=z0g[:], in_=dram['z0a'][:, bass.ds(i * 24, 24)])
                    emit_step2(nc, st, z0g[:], out_s[:, bass.ds(i, 1)], whh1_fn)
                # close dangling z0ps groups from the final iteration
                nc.vector.tensor_copy(st.za[:], st.z0psA[:, 0:48:2])
                nc.vector.tensor_copy(st.za[:], st.z0psA[:, 1:48:2])
                nc.vector.tensor_copy(st.za[:], st.z0psB[:, 0:48:2])
                nc.vector.tensor_copy(st.za[:], st.z0psB[:, 1:48:2])
            nc.sync.dma_start(out=out_d[:], in_=out_s[:])
    nc.compile()
    return nc


def host_pack_all(inputs):
    hp = HostPack2(inputs)
    cp = ConvPack(inputs)
    b0_cm = col_major(hp.b0, 24)
    sim = step_input_maps2(hp)
    wa_img = ConvPack.wa_stream_img(hp.Wa)
    blob_parts = {nm: hp.imgs[nm] for nm in ('whh0', 'wp', 'wih1', 'whh1', 'post')}
    blob_parts['fcws'] = cp.fcw_img
    blob_parts['was'] = wa_img
    blob = np.concatenate([np.ascontiguousarray(blob_parts[nm], dtype=np.float16)
                           for nm, _ in BLOB_SPEC], axis=1)
    base = {k: v for k, v in sim.items()
            if k in ('b1c', 'postbc', 'revic', 'clsc',
                     'embc_hi0', 'embc_hi1', 'embc_lo0', 'embc_lo1')}
    return hp, cp, b0_cm, base, blob


_NC_CACHE = {}
_PACK_CACHE = {}


def kernel(**inputs):
    """Full-input entry: shards batch across 8 cores, runs the Bass kernel,
    returns [8, 512, 88] float32."""
    inputs = {k: np.asarray(v) for k, v in inputs.items()}
    mel = np.asarray(inputs['mel'], np.float32)
    B = mel.shape[0]
    pk = (inputs['w_ih0'].tobytes()[:64], inputs['fc_w'].tobytes()[:64])
    if _PACK_CACHE.get('key') != pk:
        _PACK_CACHE['key'] = pk
        _PACK_CACHE['val'] = host_pack_all(inputs)
    hp, cp, b0_cm, base, blob = _PACK_CACHE['val']
    if 'nc' not in _NC_CACHE:
        _NC_CACHE['nc'] = build_nc()
    nc = _NC_CACHE['nc']
    in_maps = []
    for b in range(B):
        m = dict(base)
        m.update(conv_input_maps(cp, None, mel[b], b0_cm))
        m['wslice'] = blob[16 * b:16 * (b + 1)]
        in_maps.append(m)
    last_exc = None
    for attempt in range(3):
        try:
            res = _bass_utils.run_bass_kernel_spmd(nc, in_maps, core_ids=list(range(B)))
            break
        except Exception as e:
            last_exc = e
            if attempt == 2:
                raise
            _time.sleep(2.0)
    out = np.stack([res.results[b]['out'].T for b in range(B)], axis=0)
    return out.astype(np.float32)


# revision 6
# speedup vs baseline: 1.1087x; 1.1087x over previous
"""Trainium2 Bass kernel for nn_AR_Transcriber: conv stack + 2-layer LSTM
AR decode, fp16 hi/lo pair arithmetic (fp32-grade), data-parallel over batch
across 8 NeuronCores."""
import numpy as np
import concourse.mybir as mybir
import concourse.bass as bass

import numpy as np
import concourse.mybir as mybir

F16 = mybir.dt.float16
F32 = mybir.dt.float32
AF = mybir.ActivationFunctionType
ALU = mybir.AluOpType
AX = mybir.AxisListType
LO = 4096.0


def split_pair(W):
    W = np.asarray(W, np.float32)
    W1 = W.astype(np.float16)
    W2 = ((W - W1.astype(np.float32)) * LO).astype(np.float16)
    return W1, W2


def pack_pair_img(W, n_k, n_m, m_width=128):
    """W [out,in] -> [128, n_m*n_k*2*m_width] fp16: block (m,k) holds
    [W1tile | W2tile] of W.T, m-major. Zero-padded."""
    out_dim, in_dim = W.shape
    W1, W2 = split_pair(W)
    img = np.zeros((128, n_m * n_k * 2 * m_width), np.float16)
    for m in range(n_m):
        for k in range(n_k):
            col = (m * n_k + k) * 2 * m_width
            r0, r1 = k * 128, min((k + 1) * 128, in_dim)
            c0, c1 = m * m_width, min((m + 1) * m_width, out_dim)
            img[0:r1 - r0, col:col + (c1 - c0)] = W1[c0:c1, r0:r1].T
            img[0:r1 - r0, col + m_width:col + m_width + (c1 - c0)] = W2[c0:c1, r0:r1].T
    return img


def col_major(v, n_m):
    return np.ascontiguousarray(np.asarray(v, np.float32).reshape(n_m, 128).T)


class HostPack2:
    def __init__(self, inputs):
        W_ih0 = np.asarray(inputs['w_ih0'], np.float32)
        self.Wa = np.ascontiguousarray(W_ih0[:, :768])
        Wp = W_ih0[:, 768:]
        Wp2 = np.zeros((3072, 256), np.float32)
        idx = np.arange(88)
        Wp2[:, idx] = Wp[:, idx * 2]            # e=0 -> k-tile 0, partitions 0..87
        Wp2[:, 128 + idx] = Wp[:, idx * 2 + 1]  # e=1 -> k-tile 1
        post = np.asarray(inputs['post_w'], np.float32).reshape(88, 5, 768)
        post_img = np.zeros((128, 5 * 6 * 2 * 88), np.float16)
        for c in range(5):
            pc = pack_pair_img(np.ascontiguousarray(post[:, c, :]), 6, 1, m_width=88)
            post_img[:, c * 1056:(c + 1) * 1056] = pc
        self.imgs = {
            'whh0': pack_pair_img(np.asarray(inputs['w_hh0'], np.float32), 6, 24),
            'wp': pack_pair_img(Wp2, 2, 24),
            'wih1': pack_pair_img(np.asarray(inputs['w_ih1'], np.float32), 6, 24),
            'whh1': pack_pair_img(np.asarray(inputs['w_hh1'], np.float32), 6, 24),
            'post': post_img,
        }
        self.b0 = (np.asarray(inputs['b_ih0'], np.float32)
                   + np.asarray(inputs['b_hh0'], np.float32)).astype(np.float32)
        self.b1_cm = col_major((np.asarray(inputs['b_ih1'], np.float32)
                                + np.asarray(inputs['b_hh1'], np.float32)), 24)
        self.postb = np.ascontiguousarray(
            np.asarray(inputs['post_b'], np.float32).reshape(88, 5))
        emb = np.asarray(inputs['emb'], np.float32)
        e1, e2 = split_pair(emb)
        self.embc = {}
        for e in range(2):
            self.embc[('hi', e)] = np.broadcast_to(
                e1[:, e].astype(np.float32), (88, 5)).copy()
            self.embc[('lo', e)] = np.broadcast_to(
                e2[:, e].astype(np.float32), (88, 5)).copy()
        self.revi = np.broadcast_to(
            np.array([4, 3, 2, 1, 0], np.float32), (88, 5)).copy()
        self.clsc = np.broadcast_to(
            np.array([0, 1, 2, 3, 4], np.float32), (88, 5)).copy()
        self.emb = emb


class StepTiles2:
    """Device tiles for the v2 AR loop (allocated once, bufs=1)."""
    def __init__(self, nc, pool, psp):
        t = lambda shape, dt, name: pool.tile(list(shape), dt, name=name)
        # resident pair weights
        self.whh0 = t((128, 36864), F16, "whh0")
        self.wp = t((128, 12288), F16, "wp")
        self.wih1 = t((128, 36864), F16, "wih1")
        self.post = t((128, 5280), F16, "post")
        # rhs tiles, col layout per k: [r2, r1, 0]
        # rz0: k=0..5 h1-pair, k=6..7 prev-pair; rz1: k=0..5 h2-pair
        self.rz0 = t((128, 24), F16, "rz0")
        self.rz1 = t((128, 18), F16, "rz1")
        # state
        self.c1 = t((128, 6), F32, "c1"); self.c2 = t((128, 6), F32, "c2")
        self.h1 = t((128, 6), F32, "h1"); self.h2 = t((128, 6), F32, "h2")
        # gate scratch
        self.za = t((128, 24), F32, "za"); self.zb = t((128, 24), F32, "zb")
        self.z0 = t((128, 24), F32, "z0"); self.z1 = t((128, 24), F32, "z1")
        self.t_if = t((128, 12), F32, "t_if"); self.sig_if = t((128, 12), F32, "sig_if")
        self.tg = t((128, 6), F32, "tg"); self.t_o = t((128, 6), F32, "t_o")
        self.sig_o = t((128, 6), F32, "sig_o")
        self.u = t((128, 6), F32, "u"); self.v = t((128, 6), F32, "v")
        self.tc_ = t((128, 6), F32, "tc_")
        # argmax scratch
        self.logits = t((88, 5), F32, "logits"); self.la = t((88, 5), F32, "la")
        self.mx = t((88, 1), F32, "mx"); self.eq = t((88, 5), F32, "eq")
        self.eq2 = t((88, 5), F32, "eq2")
        self.wrv = t((88, 5), F32, "wrv"); self.rr = t((88, 1), F32, "rr")
        self.idx = t((88, 1), F32, "idx"); self.ps = t((88, 1), F32, "ps")
        # consts
        self.b1c = t((128, 24), F32, "b1c")
        self.postbc = t((88, 5), F32, "postbc")
        self.revic = t((88, 5), F32, "revic")
        self.clscc = t((88, 5), F32, "clscc")
        self.embc = {(h, e): t((88, 5), F32, f"embc_{h}{e}")
                     for h in ('hi', 'lo') for e in range(2)}
        # psums: interleaved [lo|hi] per m; one accumulation group open per
        # bank at any time (per-m groups open/close contiguously per tile)
        self.z0psA = psp.tile([128, 48], F32, name="z0psA")   # whh0 part
        self.z0psB = psp.tile([128, 48], F32, name="z0psB")   # wp part
        self.z1psA = psp.tile([128, 48], F32, name="z1psA")   # whh1 part
        self.z1psB = psp.tile([128, 48], F32, name="z1psB")   # wih1 part
        self.lgps = psp.tile([88, 10], F32, name="lgps")


def _mm_pair(nc, ps_pair, img, blk_col, rz, k3, start, stop, m_width=128):
    """Two N=2 matmuls for one (m,k) pair-tile into ps_pair [P,2]=[lo,hi].
    W1 reads rz[k3:k3+2]=[r2,r1] -> [lo+=W1@r2, hi+=W1@r1];
    W2 reads rz[k3+1:k3+3]=[r1,0] -> [lo+=W2@r1, hi+=0]."""
    W1 = img[:, blk_col:blk_col + m_width]
    W2 = img[:, blk_col + m_width:blk_col + 2 * m_width]
    nc.tensor.matmul(ps_pair, W1, rz[:, k3:k3 + 2], start=start, stop=False)
    nc.tensor.matmul(ps_pair, W2, rz[:, k3 + 1:k3 + 3], start=False, stop=stop)


def emit_A(nc, st):
    """z0 whh0-part for the NEXT step -> z0psA (per-m group opens+closes)."""
    for m in range(24):
        pp = st.z0psA[:, 2 * m:2 * m + 2]
        for k in range(6):
            _mm_pair(nc, pp, st.whh0, (m * 6 + k) * 256, st.rz0, 3 * k,
                     start=(k == 0), stop=(k == 5))


def emit_A_wp(nc, st):
    """z0 wp-part for the NEXT step -> z0psB (per-m group opens+closes)."""
    for m in range(24):
        pp = st.z0psB[:, 2 * m:2 * m + 2]
        for k in range(2):
            _mm_pair(nc, pp, st.wp, (m * 2 + k) * 256, st.rz0, 3 * (6 + k),
                     start=(k == 0), stop=(k == 1))


def _emit_gate_tail(nc, st, z, c, h, rz):
    """Common gates: z [128,24] -> h, c updated; h pair into rz cols."""
    TT, TS = nc.vector.tensor_tensor, nc.vector.tensor_scalar
    nc.scalar.activation(st.t_if[:], z[:, 0:12], AF.Tanh, scale=0.5)
    TS(st.sig_if[:], st.t_if[:], 0.5, 0.5, ALU.mult, ALU.add)
    nc.scalar.activation(st.tg[:], z[:, 12:18], AF.Tanh)
    nc.scalar.activation(st.t_o[:], z[:, 18:24], AF.Tanh, scale=0.5)
    TS(st.sig_o[:], st.t_o[:], 0.5, 0.5, ALU.mult, ALU.add)
    TT(st.u[:], st.sig_if[:, 6:12], c[:], ALU.mult)
    TT(st.v[:], st.sig_if[:, 0:6], st.tg[:], ALU.mult)
    TT(c[:], st.u[:], st.v[:], ALU.add)
    nc.scalar.activation(st.tc_[:], c[:], AF.Tanh)
    TT(h[:], st.sig_o[:], st.tc_[:], ALU.mult)
    nc.vector.tensor_copy(rz[:, 1:18:3], h[:])
    TT(st.u[:], h[:], rz[:, 1:18:3], ALU.subtract)
    TS(rz[:, 0:18:3], st.u[:], LO, None, ALU.mult)


def emit_step2(nc, st, z0g_ap, out_col_ap, whh1_fn, last=False):
    """One software-pipelined AR iteration: gates/argmax of step i + z0 GEMVs
    for step i+1. whh1_fn(m) -> [128, 1536] fp16 tile ([W1|W2] x 6k)."""
    TT, TS = nc.vector.tensor_tensor, nc.vector.tensor_scalar

    # [DVE] gates0(i): combine z0psA/B pairs + z0a(+b0) -> h1(i) into rz0
    # (each op reads at most one PSUM input)
    TS(st.za[:], st.z0psA[:, 0:48:2], 1.0 / LO, None, ALU.mult)
    TT(st.za[:], st.za[:], st.z0psA[:, 1:48:2], ALU.add)
    TS(st.zb[:], st.z0psB[:, 0:48:2], 1.0 / LO, None, ALU.mult)
    TT(st.zb[:], st.zb[:], st.z0psB[:, 1:48:2], ALU.add)
    TT(st.za[:], st.za[:], z0g_ap, ALU.add)
    TT(st.z0[:], st.za[:], st.zb[:], ALU.add)
    _emit_gate_tail(nc, st, st.z0, st.c1, st.h1, st.rz0)

    # [PE] z1 = whh1 @ h2(i-1) + wih1 @ h1(i); one psum group per m.
    LEAD = 3
    def whh1_grp(m):
        g = whh1_fn(m)
        pp = st.z1psA[:, 2 * m:2 * m + 2]
        for k in range(6):
            _mm_pair(nc, pp, g, k * 256, st.rz1, 3 * k,
                     start=(k == 0), stop=(k == 5))

    def wih1_grp(m):
        pp = st.z1psB[:, 2 * m:2 * m + 2]
        for k in range(6):
            _mm_pair(nc, pp, st.wih1, (m * 6 + k) * 256, st.rz0, 3 * k,
                     start=(k == 0), stop=(k == 5))

    for m in range(LEAD):
        whh1_grp(m)
    for m in range(24):
        if m + LEAD < 24:
            whh1_grp(m + LEAD)
        wih1_grp(m)

    # [DVE] gates1(i): combine z1psA/B pairs + b1 -> h2(i) into rz1
    TS(st.za[:], st.z1psA[:, 0:48:2], 1.0 / LO, None, ALU.mult)
    TT(st.za[:], st.za[:], st.z1psA[:, 1:48:2], ALU.add)
    TS(st.zb[:], st.z1psB[:, 0:48:2], 1.0 / LO, None, ALU.mult)
    TT(st.zb[:], st.zb[:], st.z1psB[:, 1:48:2], ALU.add)
    TT(st.za[:], st.za[:], st.b1c[:], ALU.add)
    TT(st.z1[:], st.za[:], st.zb[:], ALU.add)
    _emit_gate_tail(nc, st, st.z1, st.c2, st.h2, st.rz1)

    # [PE] A-whh0 for step i+1 (overlaps gates1 completion)
    if not last:
        emit_A(nc, st)

    # [PE] logits: post @ h2-pair
    for c in range(5):
        pp = st.lgps[:, 2 * c:2 * c + 2]
        for k in range(6):
            _mm_pair(nc, pp, st.post, (c * 6 + k) * 176, st.rz1, 3 * k,
                     start=(k == 0), stop=(k == 5), m_width=88)

    # [DVE] argmax + prev-pair into rz0
    TS(st.la[:], st.lgps[:, 0:10:2], 1.0 / LO, None, ALU.mult)
    TT(st.la[:], st.la[:], st.lgps[:, 1:10:2], ALU.add)
    TT(st.logits[:], st.la[:], st.postbc[:], ALU.add)
    nc.vector.reduce_max(st.mx[:], st.logits[:], axis=AX.X)
    TS(st.eq[:], st.logits[:], st.mx[:, 0:1], None, ALU.is_equal)
    TT(st.wrv[:], st.eq[:], st.revic[:], ALU.mult)
    nc.vector.reduce_max(st.rr[:], st.wrv[:], axis=AX.X)
    TS(st.idx[:], st.rr[:], -1.0, 4.0, ALU.mult, ALU.add)
    nc.vector.tensor_copy(out_col_ap, st.idx[:])
    TS(st.eq2[:], st.clscc[:], st.idx[:, 0:1], None, ALU.is_equal)  # tie-proof
    for e in range(2):
        TT(st.wrv[:], st.eq2[:], st.embc[('hi', e)][:], ALU.mult)
        nc.vector.reduce_sum(st.ps[:], st.wrv[:], axis=AX.X)
        nc.vector.tensor_copy(st.rz0[0:88, 3 * (6 + e) + 1:3 * (6 + e) + 2], st.ps[:])
        TT(st.wrv[:], st.eq2[:], st.embc[('lo', e)][:], ALU.mult)
        nc.vector.reduce_sum(st.ps[:], st.wrv[:], axis=AX.X)
        nc.vector.tensor_copy(st.rz0[0:88, 3 * (6 + e):3 * (6 + e) + 1], st.ps[:])

    # [PE] A-wp for step i+1 (needs prev(i))
    if not last:
        emit_A_wp(nc, st)


def load_step_consts2(nc, st, dram):
    nc.sync.dma_start(out=st.whh0[:], in_=dram['whh0'][:])
    nc.sync.dma_start(out=st.wp[:], in_=dram['wp'][:])
    nc.sync.dma_start(out=st.wih1[:], in_=dram['wih1'][:])
    nc.sync.dma_start(out=st.post[:], in_=dram['post'][:])
    nc.sync.dma_start(out=st.b1c[:], in_=dram['b1c'][:])
    nc.sync.dma_start(out=st.postbc[:], in_=dram['postbc'][:])
    nc.sync.dma_start(out=st.revic[:], in_=dram['revic'][:])
    nc.sync.dma_start(out=st.clscc[:], in_=dram['clsc'][:])
    for h in ('hi', 'lo'):
        for e in range(2):
            nc.sync.dma_start(out=st.embc[(h, e)][:], in_=dram[f'embc_{h}{e}'][:])
    for tile in (st.c1, st.c2, st.rz0, st.rz1, st.h1, st.h2):
        nc.vector.memset(tile[:], 0.0)


def step_input_maps2(hp):
    m = {nm: hp.imgs[nm] for nm in ('whh0', 'wp', 'wih1', 'post', 'whh1')}
    m['b1c'] = hp.b1_cm
    m['postbc'] = hp.postb
    m['revic'] = hp.revi
    m['clsc'] = hp.clsc
    for h in ('hi', 'lo'):
        for e in range(2):
            m[f'embc_{h}{e}'] = hp.embc[(h, e)]
    return m


# ============== conv/fc/z0a ==============

BN_EPS = 1e-5
T = 512
TC = 16          # t-rows per chunk
NCH = T // TC    # 32 chunks


def fold_bn(cw, cb, g, b, m, v):
    scale = (np.asarray(g, np.float32) / np.sqrt(np.asarray(v, np.float32) + np.float32(BN_EPS))).astype(np.float32)
    w = (np.asarray(cw, np.float32) * scale[:, None, None, None]).astype(np.float32)
    bias = (np.asarray(cb, np.float32) * scale + np.asarray(b, np.float32)
            - np.asarray(m, np.float32) * scale).astype(np.float32)
    return w, bias


class ConvPack:
    def __init__(self, inputs):
        w1, b1 = fold_bn(inputs['conv1_w'], inputs['conv1_b'], inputs['bn1_g'],
                         inputs['bn1_b'], inputs['bn1_m'], inputs['bn1_v'])
        w2, b2 = fold_bn(inputs['conv2_w'], inputs['conv2_b'], inputs['bn2_g'],
                         inputs['bn2_b'], inputs['bn2_m'], inputs['bn2_v'])
        w3, b3 = fold_bn(inputs['conv3_w'], inputs['conv3_b'], inputs['bn3_g'],
                         inputs['bn3_b'], inputs['bn3_m'], inputs['bn3_v'])
        c1l = np.zeros((9, 48), np.float32)
        for dt in range(3):
            for df in range(3):
                c1l[3*dt+df] = w1[:, 0, dt, df]
        self.c1l_1, self.c1l_2 = split_pair(c1l)
        self.b1 = np.ascontiguousarray(b1.reshape(48, 1))
        c2a = np.zeros((48, 9 * 48), np.float16); c2b = np.zeros((48, 9 * 48), np.float16)
        c3a = np.zeros((48, 9 * 96), np.float16); c3b = np.zeros((48, 9 * 96), np.float16)
        for tap in range(9):
            dt, df = tap // 3, tap % 3
            a, bq = split_pair(np.ascontiguousarray(w2[:, :, dt, df].T))
            c2a[:, tap*48:(tap+1)*48] = a; c2b[:, tap*48:(tap+1)*48] = bq
            a, bq = split_pair(np.ascontiguousarray(w3[:, :, dt, df].T))
            c3a[:, tap*96:(tap+1)*96] = a; c3b[:, tap*96:(tap+1)*96] = bq
        self.c2l_1, self.c2l_2 = c2a, c2b
        self.c3l_1, self.c3l_2 = c3a, c3b
        self.b2 = np.ascontiguousarray(b2.reshape(48, 1))
        self.b3 = np.ascontiguousarray(b3.reshape(96, 1))
        fcw = np.asarray(inputs['fc_w'], np.float32)
        fcw2 = np.zeros((768, 57 * 128), np.float32)
        for f in range(57):
            fcw2[:, f * 128: f * 128 + 96] = fcw[:, np.arange(96) * 57 + f]
        img = np.zeros((128, 57 * 6 * 256), np.float16)
        W1, W2 = split_pair(fcw2)
        for f in range(57):
            for m in range(6):
                col = (f * 6 + m) * 256
                img[:, col:col+128] = W1[m*128:(m+1)*128, f*128:(f+1)*128].T
                img[:, col+128:col+256] = W2[m*128:(m+1)*128, f*128:(f+1)*128].T
        self.fcw_img = img
        self.fcb_pm = np.ascontiguousarray(np.asarray(inputs['fc_b'], np.float32).reshape(6, 128).T)

    @staticmethod
    def wa_stream_img(Wa):
        W1, W2 = split_pair(Wa)
        img = np.zeros((128, 24 * 6 * 256), np.float16)
        for m in range(24):
            for k in range(6):
                col = (m * 6 + k) * 256
                img[:, col:col+128] = W1[m*128:(m+1)*128, k*128:(k+1)*128].T
                img[:, col+128:col+256] = W2[m*128:(m+1)*128, k*128:(k+1)*128].T
        return img


def melpad_pair(mel_row):
    mp = np.zeros((514, 231), np.float32)
    mp[1:513, 1:230] = np.asarray(mel_row, np.float32)
    m1 = mp.astype(np.float16)
    m2 = ((mp - m1.astype(np.float32)) * LO).astype(np.float16)
    return m1, m2


def emit_zero_pads(nc, pool, scrs):
    """Zero pad borders of HBM scratches [(handle, C, H, W), ...]."""
    mx = max(max(h, w) for _, _, h, w in scrs)
    zt = pool.tile([128, mx], F16, name="zpad")
    nc.vector.memset(zt[:], 0.0)
    for scr, C, H, W in scrs:
        nc.sync.dma_start(out=scr[:, 0, :], in_=zt[:C, :W])
        nc.sync.dma_start(out=scr[:, H-1, :], in_=zt[:C, :W])
        nc.sync.dma_start(out=scr[:, :, 0], in_=zt[:C, :H])
        nc.sync.dma_start(out=scr[:, :, W-1], in_=zt[:C, :H])


def emit_conv1(nc, tc, pool, psp, dram):
    Fp = 231
    c1w1 = pool.tile([9, 48], F16, name="c1w1"); c1w2 = pool.tile([9, 48], F16, name="c1w2")
    b1t = pool.tile([48, 1], F32, name="b1t")
    nc.sync.dma_start(out=c1w1[:], in_=dram['c1l_1'][:])
    nc.sync.dma_start(out=c1w2[:], in_=dram['c1l_2'][:])
    nc.sync.dma_start(out=b1t[:], in_=dram['c1b'][:])
    NW = TC * Fp
    with tc.For_i(0, NCH, 1, name="c1loop") as ch:
        t0r = ch * TC
        P1 = pool.tile([9, NW], F16, name="P1", bufs=2)
        P2 = pool.tile([9, NW], F16, name="P2", bufs=2)
        for tap in range(9):
            dt, df = tap // 3, tap % 3
            w = Fp - df
            nc.sync.dma_start(
                out=P1[tap:tap+1, :].rearrange("a (i j) -> a i j", j=Fp)[:, :, 0:w],
                in_=dram['mel1'][bass.ds(t0r + dt, TC), df:Fp])
            nc.sync.dma_start(
                out=P2[tap:tap+1, :].rearrange("a (i j) -> a i j", j=Fp)[:, :, 0:w],
                in_=dram['mel2'][bass.ds(t0r + dt, TC), df:Fp])
        for wi in range(TC // 2):
            off = wi * 2 * Fp
            N = 2 * Fp
            ph = psp.tile([48, 462], F32, name="c1ph", bufs=2)
            pl = psp.tile([48, 462], F32, name="c1pl", bufs=2)
            nc.tensor.matmul(ph[:, :N], c1w1[:], P1[:, off:off+N], start=True, stop=True)
            nc.tensor.matmul(pl[:, :N], c1w1[:], P2[:, off:off+N], start=True, stop=False)
            nc.tensor.matmul(pl[:, :N], c1w2[:], P1[:, off:off+N], start=False, stop=True)
            mg = pool.tile([48, 462], F32, name="c1mg", bufs=2)
            nc.vector.tensor_scalar(mg[:, :N], pl[:, :N], 1.0 / LO, None, ALU.mult)
            nc.vector.tensor_tensor(mg[:, :N], mg[:, :N], ph[:, :N], ALU.add)
            rl = pool.tile([48, 462], F32, name="c1rl", bufs=2)
            nc.scalar.activation(rl[:, :N], mg[:, :N], AF.Relu, bias=b1t[:, 0:1])
            s1 = pool.tile([48, 462], F16, name="c1s1", bufs=2)
            s2 = pool.tile([48, 462], F16, name="c1s2", bufs=2)
            nc.vector.tensor_copy(s1[:, :N], rl[:, :N])
            nc.vector.tensor_tensor(mg[:, :N], rl[:, :N], s1[:, :N], ALU.subtract)
            nc.vector.tensor_scalar(s2[:, :N], mg[:, :N], LO, None, ALU.mult)
            for s, nm in ((s1, 'c1p1'), (s2, 'c1p2')):
                nc.sync.dma_start(
                    out=dram[nm][:, bass.ds(t0r + wi * 2 + 1, 2), 1:230],
                    in_=s.rearrange("c (i j) -> c i j", j=Fp)[:, 0:2, 0:229])


def emit_convN(nc, tc, pool, psp, dram, in_nm, wkey, Cin, Cout, Fin, sink):
    """conv2/3: input HBM pad-pair [Cin, 514, Fin+2]; 27 MMs per 2-row window;
    relu; pool w2; sink(nc, ch_reg, wi, po_view [Cout,2,Fo])."""
    Fp = Fin + 2
    N = 2 * Fp
    Fo = Fin // 2
    wt1 = pool.tile([48, 9 * Cout], F16, name=f"wt1{wkey}")
    wt2 = pool.tile([48, 9 * Cout], F16, name=f"wt2{wkey}")
    bt = pool.tile([Cout, 1], F32, name=f"bt{wkey}")
    nc.sync.dma_start(out=wt1[:Cin, :], in_=dram[wkey + '_1'][:])
    nc.sync.dma_start(out=wt2[:Cin, :], in_=dram[wkey + '_2'][:])
    nc.sync.dma_start(out=bt[:], in_=dram[wkey + 'b'][:])
    with tc.For_i(0, NCH, 1, name=f"loop{wkey}") as ch:
        t0r = ch * TC
        X1 = pool.tile([Cin, (TC + 2) * Fp + 2], F16, name="cnX1", bufs=2)
        X2 = pool.tile([Cin, (TC + 2) * Fp + 2], F16, name="cnX2", bufs=2)
        nc.sync.dma_start(out=X1[:, 0:(TC + 2) * Fp].rearrange("c (i j) -> c i j", j=Fp),
                          in_=dram[in_nm + '1'][:, bass.ds(t0r, TC + 2), :])
        nc.sync.dma_start(out=X2[:, 0:(TC + 2) * Fp].rearrange("c (i j) -> c i j", j=Fp),
                          in_=dram[in_nm + '2'][:, bass.ds(t0r, TC + 2), :])
        for wi in range(TC // 2):
            ph = psp.tile([Cout, 512], F32, name="cnph", bufs=2)
            pl = psp.tile([Cout, 512], F32, name="cnpl", bufs=2)
            for tap in range(9):
                dt, df = tap // 3, tap % 3
                off = (wi * 2 + dt) * Fp + df
                l1 = wt1[:Cin, tap*Cout:(tap+1)*Cout]
                l2 = wt2[:Cin, tap*Cout:(tap+1)*Cout]
                nc.tensor.matmul(ph[:, :N], l1, X1[:, off:off+N], start=(tap == 0), stop=(tap == 8))
                nc.tensor.matmul(pl[:, :N], l1, X2[:, off:off+N], start=(tap == 0), stop=False)
                nc.tensor.matmul(pl[:, :N], l2, X1[:, off:off+N], start=False, stop=(tap == 8))
            mg = pool.tile([Cout, 512], F32, name="cnmg", bufs=2)
            nc.vector.tensor_scalar(mg[:, :N], pl[:, :N], 1.0 / LO, None, ALU.mult)
            nc.vector.tensor_tensor(mg[:, :N], mg[:, :N], ph[:, :N], ALU.add)
            rl = pool.tile([Cout, 512], F32, name="cnrl", bufs=2)
            nc.scalar.activation(rl[:, :N], mg[:, :N], AF.Relu, bias=bt[:, 0:1])
            pv = rl[:, :N].rearrange("c (i j) -> c i j", j=Fp)
            po = pool.tile([Cout, 2 * Fo], F32, name="cnpo", bufs=2)
            pov = po.rearrange("c (i j) -> c i j", j=Fo)
            nc.vector.tensor_tensor(pov, pv[:, 0:2, 0:2*Fo:2], pv[:, 0:2, 1:1+2*Fo:2], ALU.max)
            sink(nc, t0r, wi, po, Fo)


def make_pad_sink(pool, dram, out_nm, Cout):
    def sink(nc, t0r, wi, po, Fo):
        N = 2 * Fo
        s1 = pool.tile([Cout, 256], F16, name="pds1", bufs=2)
        s2 = pool.tile([Cout, 256], F16, name="pds2", bufs=2)
        tmp = pool.tile([Cout, 256], F32, name="pdtmp", bufs=2)
        nc.vector.tensor_copy(s1[:, :N], po[:])
        nc.vector.tensor_tensor(tmp[:, :N], po[:], s1[:, :N], ALU.subtract)
        nc.vector.tensor_scalar(s2[:, :N], tmp[:, :N], LO, None, ALU.mult)
        nc.sync.dma_start(out=dram[out_nm + '1'][:, bass.ds(t0r + wi*2 + 1, 2), 1:1+Fo],
                          in_=s1[:, :N].rearrange("c (i j) -> c i j", j=Fo))
        nc.sync.dma_start(out=dram[out_nm + '2'][:, bass.ds(t0r + wi*2 + 1, 2), 1:1+Fo],
                          in_=s2[:, :N].rearrange("c (i j) -> c i j", j=Fo))
    return sink


def make_feat_sink(pool, feat1, feat2, Cout):
    def sink(nc, t0r, wi, po, Fo):
        N = 2 * Fo
        s1 = pool.tile([Cout, N], F16, name="fts1", bufs=2)
        s2 = pool.tile([Cout, N], F16, name="fts2", bufs=2)
        tmp = pool.tile([Cout, N], F32, name="fttmp", bufs=2)
        nc.vector.tensor_copy(s1[:], po[:, :N])
        nc.vector.tensor_tensor(tmp[:], po[:, :N], s1[:], ALU.subtract)
        nc.vector.tensor_scalar(s2[:], tmp[:], LO, None, ALU.mult)
        for s, ft in ((s1, feat1), (s2, feat2)):
            nc.vector.tensor_copy(
                ft.rearrange("c (f t) -> c f t", t=512)[0:Cout, :, bass.ds(t0r + wi*2, 2)],
                s.rearrange("c (i j) -> c j i", j=Fo))
    return sink


def emit_fc_z0a(nc, tc, pool, psp, spool, dram, feat1, feat2, b0c, ac1, ac2):
    fcbt = pool.tile([128, 6], F32, name="fcbt")
    nc.sync.dma_start(out=fcbt[:], in_=dram['fcb'][:])
    for half in range(2):
        for mi in range(3):
            m = half * 3 + mi
            ph = psp.tile([128, 512], F32, name=f"fch{mi}")
            pl = psp.tile([128, 512], F32, name=f"fcl{mi}")
            for f in range(57):
                g = spool.tile([128, 256], F16, name="fcg")
                nc.sync.dma_start(out=g[:], in_=dram['fcws'][:, (f*6+m)*256:(f*6+m+1)*256])
                nc.tensor.matmul(ph[:], g[:, 0:128], feat1[:, f*512:(f+1)*512],
                                 start=(f == 0), stop=(f == 56))
                nc.tensor.matmul(pl[:], g[:, 0:128], feat2[:, f*512:(f+1)*512],
                                 start=(f == 0), stop=False)
                nc.tensor.matmul(pl[:], g[:, 128:256], feat1[:, f*512:(f+1)*512],
                                 start=False, stop=(f == 56))
            mg = pool.tile([128, 512], F32, name="fcmg", bufs=2)
            nc.vector.tensor_scalar(mg[:], pl[:], 1.0 / LO, None, ALU.mult)
            nc.vector.tensor_tensor(mg[:], mg[:], ph[:], ALU.add)
            nc.vector.tensor_scalar(mg[:], mg[:], fcbt[:, m:m+1], None, ALU.add)
            nc.vector.tensor_copy(ac1[:, m*512:(m+1)*512], mg[:])
            nc.vector.tensor_tensor(mg[:], mg[:], ac1[:, m*512:(m+1)*512], ALU.subtract)
            nc.vector.tensor_scalar(ac2[:, m*512:(m+1)*512], mg[:], LO, None, ALU.mult)
    for m in range(24):
        ph = psp.tile([128, 512], F32, name="zah")
        pl = psp.tile([128, 512], F32, name="zal")
        for k in range(6):
            g = spool.tile([128, 256], F16, name="wag")
            nc.sync.dma_start(out=g[:], in_=dram['was'][:, (m*6+k)*256:(m*6+k+1)*256])
            nc.tensor.matmul(ph[:], g[:, 0:128], ac1[:, k*512:(k+1)*512],
                             start=(k == 0), stop=(k == 5))
            nc.tensor.matmul(pl[:], g[:, 0:128], ac2[:, k*512:(k+1)*512],
                             start=(k == 0), stop=False)
            nc.tensor.matmul(pl[:], g[:, 128:256], ac1[:, k*512:(k+1)*512],
                             start=False, stop=(k == 5))
        mg = pool.tile([128, 512], F32, name="zamg", bufs=2)
        nc.vector.tensor_scalar(mg[:], pl[:], 1.0 / LO, None, ALU.mult)
        nc.vector.tensor_tensor(mg[:], mg[:], ph[:], ALU.add)
        nc.vector.tensor_scalar(mg[:], mg[:], b0c[:, m:m+1], None, ALU.add)
        nc.sync.dma_start(
            out=dram['z0a'].rearrange("p (t q) -> p t q", q=24)[:, 0:512, m],
            in_=mg[:])


def conv_input_maps(cp, wa_img, mel_row, b0_cm):
    m1, m2 = melpad_pair(mel_row)
    return {
        'mel1': m1, 'mel2': m2, 'b0c': b0_cm,
        'c1l_1': cp.c1l_1, 'c1l_2': cp.c1l_2, 'c1b': cp.b1,
        'c2_1': cp.c2l_1, 'c2_2': cp.c2l_2, 'c2b': cp.b2,
        'c3_1': cp.c3l_1, 'c3_2': cp.c3l_2, 'c3b': cp.b3,
        'fcb': cp.fcb_pm,
    }


def declare_conv_drams(nc, scratch_kind="Internal"):
    """Input + scratch DRAM tensors for the conv phase. Returns dict."""
    d = {}
    ins = [('mel1', [514, 231], F16), ('mel2', [514, 231], F16),
           ('c1l_1', [9, 48], F16), ('c1l_2', [9, 48], F16), ('c1b', [48, 1], F32),
           ('c2_1', [48, 9*48], F16), ('c2_2', [48, 9*48], F16), ('c2b', [48, 1], F32),
           ('c3_1', [48, 9*96], F16), ('c3_2', [48, 9*96], F16), ('c3b', [96, 1], F32),
           ('fcb', [128, 6], F32), ('b0c', [128, 24], F32)]
    for nm, shp, dt in ins:
        d[nm] = nc.dram_tensor(nm, shp, dt, kind="ExternalInput")
    scr = [('c1p1', [48, 514, 231], F16), ('c1p2', [48, 514, 231], F16),
           ('c2p1', [48, 514, 116], F16), ('c2p2', [48, 514, 116], F16)]
    for nm, shp, dt in scr:
        d[nm] = nc.dram_tensor(nm, shp, dt, kind=scratch_kind)
    return d


def emit_conv_phase(nc, tc, dram, b0_cm):
    """Full conv+fc+z0a phase. b0_cm: [128, 24] host array for b0 (loaded here).
    Uses its own pools; returns nothing (z0a left in dram['z0a'])."""
    with tc.tile_pool(name="cvp", bufs=1) as pool, \
         tc.tile_pool(name="cvps", bufs=1, space="PSUM") as psp:
        emit_zero_pads(nc, pool, [(dram['c1p1'], 48, 514, 231), (dram['c1p2'], 48, 514, 231),
                                  (dram['c2p1'], 48, 514, 116), (dram['c2p2'], 48, 514, 116)])
        emit_conv1(nc, tc, pool, psp, dram)
        emit_convN(nc, tc, pool, psp, dram, 'c1p', 'c2', 48, 48, 229,
                   make_pad_sink(pool, dram, 'c2p', 48))
    with tc.tile_pool(name="cvp3", bufs=1) as pool:
        feat1 = pool.tile([128, 57*512], F16, name="feat1")
        feat2 = pool.tile([128, 57*512], F16, name="feat2")
        nc.vector.memset(feat1[:], 0.0)
        nc.vector.memset(feat2[:], 0.0)
        with tc.tile_pool(name="cvps3", bufs=1, space="PSUM") as psp3:
            emit_convN(nc, tc, pool, psp3, dram, 'c2p', 'c3', 48, 96, 114,
                       make_feat_sink(pool, feat1, feat2, 96))
        with tc.tile_pool(name="fcps", bufs=1, space="PSUM") as pspf, \
             tc.tile_pool(name="fcsp", bufs=4) as spool:
            b0ct = pool.tile([128, 24], F32, name="b0ct")
            nc.sync.dma_start(out=b0ct[:], in_=dram['b0c'][:])
            ac1 = pool.tile([128, 6*512], F16, name="ac1")
            ac2 = pool.tile([128, 6*512], F16, name="ac2")
            emit_fc_z0a(nc, tc, pool, pspf, spool, dram, feat1, feat2, b0ct, ac1, ac2)


# ====================== kernel assembly ======================
import time as _time
import concourse.bacc as _bacc
from concourse.tile import TileContext as _TileContext
from concourse import bass_utils as _bass_utils

_T = 512
NRES = 2

BLOB_SPEC = [('whh0', 36864), ('wp', 12288), ('wih1', 36864), ('whh1', 36864),
             ('post', 5280), ('fcws', 57 * 6 * 256), ('was', 24 * 6 * 256)]
BLOB_COLS = sum(n for _, n in BLOB_SPEC)


def build_nc(conv_loops=1, ar_loops=1):
    nc = _bacc.Bacc(num_devices=8)
    dram = declare_conv_drams(nc)
    dram['z0a'] = nc.dram_tensor('z0a', [128, 513 * 24], F32)
    sim_shapes = {
        'b1c': [128, 24], 'postbc': [88, 5], 'revic': [88, 5], 'clsc': [88, 5],
        'embc_hi0': [88, 5], 'embc_hi1': [88, 5],
        'embc_lo0': [88, 5], 'embc_lo1': [88, 5],
    }
    for k, shp in sim_shapes.items():
        dram[k] = nc.dram_tensor(k, shp, F32, kind="ExternalInput")
    wslice = nc.dram_tensor('wslice', [16, BLOB_COLS], F16, kind="ExternalInput")
    wbounce = nc.dram_tensor('wbounce', [16, BLOB_COLS], F16)
    wblob = nc.dram_tensor('wblob', [128, BLOB_COLS], F16, addr_space="Shared")
    off = 0
    for nm, ncols in BLOB_SPEC:
        dram[nm] = wblob[:, off:off + ncols]
        off += ncols
    out_d = nc.dram_tensor('out', [88, _T], F32, kind="ExternalOutput")

    with _TileContext(nc) as tc:
        nc.sync.dma_start(out=wbounce[:], in_=wslice[:])
        nc.gpsimd.collective_compute(
            "AllGather", mybir.AluOpType.bypass,
            ins=[wbounce[:]], outs=[wblob[:]],
            replica_groups=[list(range(8))])
        for _ in range(conv_loops):
            emit_conv_phase(nc, tc, dram, None)
        with tc.tile_pool(name="arp", bufs=1) as pool, \
             tc.tile_pool(name="arsp", bufs=4) as spool, \
             tc.tile_pool(name="arzp", bufs=2) as zpool, \
             tc.tile_pool(name="arps", bufs=1, space="PSUM") as psp:
            st = StepTiles2(nc, pool, psp)
            out_s = pool.tile([88, _T], F32, name="out_s")
            whh1r = pool.tile([128, NRES * 1536], F16, name="whh1r")
            load_step_consts2(nc, st, dram)
            nc.sync.dma_start(out=whh1r[:], in_=dram['whh1'][:, 0:NRES * 1536])

            def whh1_fn(m):
                if m < NRES:
                    return whh1r[:, m * 1536:(m + 1) * 1536]
                g = spool.tile([128, 1536], F16, name="whh1g")
                nc.sync.dma_start(out=g[:], in_=dram['whh1'][:, m * 1536:(m + 1) * 1536])
                return g[:]

            for rep in range(ar_loops):
                emit_A(nc, st)
                emit_A_wp(nc, st)
                with tc.For_i(0, _T, 1, hint_engines=(mybir.EngineType.PE,)) as i:
                    z0g = zpool.tile([128, 24], F32, name="z0g")
                    nc.sync.dma_start(out=z0g[:], in_=dram['z0a'][:, bass.ds(i * 24, 24)])
                    emit_step2(nc, st, z0g[:], out_s[:, bass.ds(i, 1)], whh1_fn)
                # close dangling z0ps groups from the final iteration
                nc.vector.tensor_copy(st.za[:], st.z0psA[:, 0:48:2])
                nc.vector.tensor_copy(st.za[:], st.z0psA[:, 1:48:2])
                nc.vector.tensor_copy(st.za[:], st.z0psB[:, 0:48:2])
                nc.vector.tensor_copy(st.za[:], st.z0psB[:, 1:48:2])
            nc.sync.dma_start(out=out_d[:], in_=out_s[:])
    nc.compile()
    return nc


def host_pack_all(inputs):
    hp = HostPack2(inputs)
    cp = ConvPack(inputs)
    b0_cm = col_major(hp.b0, 24)
    sim = step_input_maps2(hp)
    wa_img = ConvPack.wa_stream_img(hp.Wa)
    blob_parts = {nm: hp.imgs[nm] for nm in ('whh0', 'wp', 'wih1', 'whh1', 'post')}
    blob_parts['fcws'] = cp.fcw_img
    blob_parts['was'] = wa_img
    blob = np.concatenate([np.ascontiguousarray(blob_parts[nm], dtype=np.float16)
                           for nm, _ in BLOB_SPEC], axis=1)
    base = {k: v for k, v in sim.items()
            if k in ('b1c', 'postbc', 'revic', 'clsc',
                     'embc_hi0', 'embc_hi1', 'embc_lo0', 'embc_lo1')}
    return hp, cp, b0_cm, base, blob


_NC_CACHE = {}
_PACK_CACHE = {}


def kernel(**inputs):
    """Full-input entry: shards batch across 8 cores, runs the Bass kernel,
    returns [8, 512, 88] float32."""
    inputs = {k: np.asarray(v) for k, v in inputs.items()}
    mel = np.asarray(inputs['mel'], np.float32)
    B = mel.shape[0]
    pk = (inputs['w_ih0'].tobytes()[:64], inputs['fc_w'].tobytes()[:64])
    if _PACK_CACHE.get('key') != pk:
        _PACK_CACHE['key'] = pk
        _PACK_CACHE['val'] = host_pack_all(inputs)
    hp, cp, b0_cm, base, blob = _PACK_CACHE['val']
    if 'nc' not in _NC_CACHE:
        _NC_CACHE['nc'] = build_nc()
    nc = _NC_CACHE['nc']
    in_maps = []
    for b in range(B):
        m = dict(base)
        m.update(conv_input_maps(cp, None, mel[b], b0_cm))
        m['wslice'] = blob[16 * b:16 * (b + 1)]
        in_maps.append(m)
    last_exc = None
    for attempt in range(3):
        try:
            res = _bass_utils.run_bass_kernel_spmd(nc, in_maps, core_ids=list(range(B)))
            break
        except Exception as e:
            last_exc = e
            if attempt == 2:
                raise
            _time.sleep(2.0)
    out = np.stack([res.results[b]['out'].T for b in range(B)], axis=0)
    return out.astype(np.float32)
